# revision 36
# baseline (speedup 1.0000x reference)
"""Trainium2 Bass kernel: fractional Brownian motion kernel layer.

K[i,j] = 0.5 * sum_d (|x_id|^p + |X2_jd|^p - |x_id - X2_jd|^p),
p = 2*softplus(log_H),  x:[2048,16], X2:[2048,16] -> K:[2048,2048] f32.

Sharding: rows of x across 8 NeuronCores (256 rows each), X2 replicated.
Per-core layout: partition = i (2 tiles of 128), free = j (2048).

Pairwise pipeline per (chunk of 4 d's, i-tile):
  DVE : diff = X2r_d - x_col      tensor_scalar subtract @ 2x fp32 (per d)
  DVE : |diff|                    tensor_scalar bitwise_and 0x7FFFFFFF (uint32 view) @ 2x
  ACT : Ln                        one instr over the whole chunk
  ACT : Exp(scale=p, bias=ln.5)   -> 0.5*|diff|^p
  DVE/GPS : acc -= pw             tensor_tensor subtract (a slice of the
                                  accumulation runs on GpSimd into its own
                                  accumulator to unload the DVE)

acc is initialized to 0.5*t1_i + 0.5*t2_j (computed compactly on device), so
after the d-loop acc IS the output slab. The 0.5 folds into the Exp bias.

All activation functions used (Exp, Ln, Abs) live in the
natural_log_exp_and_others table set; get_activation_tables is narrowed so
bacc's table-load pass picks that single set (otherwise it ping-pongs
between exp_and_others and natural_log, reloading tables 19x per launch).
"""

from contextlib import ExitStack

import numpy as np

import concourse.bass as bass
import concourse.tile as tile
from concourse import mybir, bacc
from concourse.bass_utils import run_bass_kernel_spmd

AF = mybir.ActivationFunctionType
OP = mybir.AluOpType
AX = mybir.AxisListType
F32 = mybir.dt.float32
U32 = mybir.dt.uint32

N, M, D = 2048, 2048, 16
NCORES = 8
NS = N // NCORES          # 256 rows of x per core
P = 128                   # SBUF partitions
NIT = NS // P             # 2 i-tiles per core
G = 2                     # d's per chunk
NCH = D // G              # chunks
# (chunk, i-tile) units whose accumulation runs on GpSimd instead of DVE,
# spread across the schedule so DVE never starves on sq-slot reuse
GPS_UNITS = set()   # GpSimd streaming contends with DVE SBUF ports (HW-measured)
# units whose |diff| is computed as ACT Square(x2r - x) instead of DVE sub+mask,
# balancing DVE vs ACT load
SQ_UNITS = set()    # ACT is the HW-binding engine; keep all abs work on DVE
LN_HALF = float(np.log(0.5))
ABS_MASK = 0x7FFFFFFF

_CACHE = {}


def _patch_act_tables():
    """Force every activation function we use into one table set so the
    act-table-load pass emits a single load."""
    if _CACHE.get("patched"):
        return
    import concourse.hw_specs as hw_specs
    import concourse.bacc as bacc_mod

    orig = hw_specs.get_activation_tables
    ours = {AF.Exp, AF.Ln, AF.Abs, AF.Square}

    def patched(module_arch):
        tabs = {k: set(v) for k, v in orig(module_arch).items()}
        for name, fns in tabs.items():
            if name != "natural_log_exp_and_others":
                fns -= ours
        return tabs

    bacc_mod.get_activation_tables = patched
    _CACHE["patched"] = True


def _build_nc(reps=1, body_reps=1):
    _patch_act_tables()
    nc = bacc.Bacc(trn_type="TRN2", target_bir_lowering=False, debug=False,
                   num_devices=NCORES)

    xsh = nc.declare_dram_parameter("xsh", [NS, D], F32, isOutput=False)
    x2t = nc.declare_dram_parameter("x2t", [D, M], F32, isOutput=False)
    x2n = nc.declare_dram_parameter("x2n", [M, D], F32, isOutput=False)
    logh = nc.declare_dram_parameter("logh", [1, 1], F32, isOutput=False)
    out = nc.declare_dram_parameter("out", [NS, M], F32, isOutput=True)
    t2scr = nc.dram_tensor("t2scr", [1, M], F32)

    xsh_ap, x2t_ap, x2n_ap, logh_ap, out_ap, t2scr_ap = (
        h.ap() for h in (xsh, x2t, x2n, logh, out, t2scr))

    with tile.TileContext(nc) as tc, ExitStack() as ctx:
        const = ctx.enter_context(tc.tile_pool(name="const", bufs=1))
        x2rp = ctx.enter_context(tc.tile_pool(name="x2r", bufs=3))
        sqp = ctx.enter_context(tc.tile_pool(name="sq", bufs=5))
        accp = ctx.enter_context(tc.tile_pool(name="acc", bufs=1))

        if reps > 1:  # benchmark mode: repeat the whole body on-device
            loop = ctx.enter_context(tc.For_i(0, reps, 1))

        for _body in range(body_reps):
            _emit_body(nc, tc, const, x2rp, sqp, accp,
                       xsh_ap, x2t_ap, x2n_ap, logh_ap, out_ap, t2scr_ap)

    nc.compile()
    return nc


def _emit_body(nc, tc, const, x2rp, sqp, accp,
               xsh_ap, x2t_ap, x2n_ap, logh_ap, out_ap, t2scr_ap):
    if True:

        # ---- tiny latency-critical loads first (sync/SP HWDGE ring) ----
        # x shard -> [128, it*16+d] (one DMA; one completion sem)
        xsb = const.tile([P, NIT * D], F32)
        nc.sync.dma_start(
            out=xsb,
            in_=bass.AP(tensor=xsh_ap.tensor, offset=0,
                        ap=[[D, P], [P * D, NIT], [1, D]]))

        if SQ_UNITS:
            xneg = const.tile([P, NIT * D], F32)
            nc.vector.tensor_scalar(out=xneg[:, :], in0=xsb[:, :],
                                    scalar1=-1.0, scalar2=None, op0=OP.mult)

        lh = const.tile([P, 1], F32)
        nc.sync.dma_start(
            out=lh,
            in_=bass.AP(tensor=logh_ap.tensor, offset=0, ap=[[0, P], [1, 1]]))

        # ---- X2 broadcast rows, one 1MB DMA per d for fine pipelining;
        # chunk 0 first so compute can start immediately ----
        x2rs = {}
        def load_chunk(ch):
            x2r = x2rp.tile([P, G * M], F32)
            for g in range(G):
                d = ch * G + g
                nc.sync.dma_start(
                    out=x2r[:, g * M:(g + 1) * M],
                    in_=bass.AP(tensor=x2t_ap.tensor, offset=d * M,
                                ap=[[0, P], [1, M]]))
            x2rs[ch] = x2r
        load_chunk(0)
        x2c = const.tile([P, (M // P) * D], F32)
        JT = M // P   # 16 j's per partition, p-major: j = p*JT + jt
        nc.sync.dma_start(
            out=x2c,
            in_=bass.AP(tensor=x2n_ap.tensor, offset=0,
                        ap=[[JT * D, P], [D, JT], [1, D]]))
        for ch in range(1, NCH):
            load_chunk(ch)

        # ---- H and p = 2H, replicated to all partitions ----
        eh = const.tile([P, 1], F32)
        nc.scalar.activation(out=eh, in_=lh, func=AF.Exp)
        hcol = const.tile([P, 1], F32)
        # ln(1 + e^logh) = softplus(logh) = H
        nc.scalar.activation(out=hcol, in_=eh, func=AF.Ln, bias=1.0)
        pcol = const.tile([P, 1], F32)
        nc.scalar.mul(pcol, hcol, 2.0)
        lnhalf = const.tile([P, 1], F32)
        nc.gpsimd.memset(lnhalf[:, :], LN_HALF)

        # ---- t1 (halved): 0.5*sum_d |x_id|^p, per-partition scalars ----
        e1 = const.tile([P, NIT * D], F32)
        nc.scalar.activation(out=e1, in_=xsb, func=AF.Abs)
        nc.scalar.activation(out=e1, in_=e1, func=AF.Ln)
        nc.scalar.activation(out=e1, in_=e1, func=AF.Exp,
                             bias=lnhalf[:, :], scale=pcol)
        t1h = const.tile([P, NIT], F32)
        nc.vector.tensor_reduce(
            out=t1h[:, :], in_=e1[:, :].rearrange("p (it d) -> p it d", it=NIT),
            axis=AX.X, op=OP.add)

        # ---- t2 (halved): compact [128 j, 16 jt x 16 d] -> row of 2048 ----
        e2 = const.tile([P, (M // P) * D], F32)
        nc.scalar.activation(out=e2, in_=x2c, func=AF.Abs)
        nc.scalar.activation(out=e2, in_=e2, func=AF.Ln)
        nc.scalar.activation(out=e2, in_=e2, func=AF.Exp,
                             bias=lnhalf[:, :], scale=pcol)
        t2c = const.tile([P, M // P], F32)
        nc.vector.tensor_reduce(
            out=t2c[:, :], in_=e2[:, :].rearrange("p (jt d) -> p jt d", d=D),
            axis=AX.X, op=OP.add)
        # roundtrip through DRAM to rearrange [p, jt] -> row j = jt*128+p,
        # then broadcast that row across all 128 partitions. Issued on the
        # scalar engine's HWDGE ring so it never blocks the x2r stream on
        # the sync ring (HWDGE DMAs are FIFO per issuing engine).
        nc.sync.dma_start(
            out=bass.AP(tensor=t2scr_ap.tensor, offset=0,
                        ap=[[JT, P], [1, JT]]),
            in_=t2c)
        t2r = const.tile([P, M], F32)
        nc.sync.dma_start(
            out=t2r,
            in_=bass.AP(tensor=t2scr_ap.tensor, offset=0, ap=[[0, P], [1, M]]))

        # ---- accumulators: zero-init (no dependency on the t2 roundtrip;
        # t1+t2 is folded in at the very end) ----
        accs, gaccs = [], {}
        acc_started = set()
        for it in range(NIT):
            acc = accp.tile([P, M], F32, tag=f"acc{it}")
            accs.append(acc)
            if any(i2 == it for (_c, i2) in GPS_UNITS):
                gacc = accp.tile([P, M], F32, tag=f"gacc{it}")
                nc.gpsimd.memset(gacc[:, :], 0.0)
                gaccs[it] = gacc

        # ---- main loop: acc += 0.5*|x_id - X2_jd|^p ----
        t12s = []
        for ch in range(NCH):
            x2r = x2rs[ch]
            if ch == NCH - 1:
                # t12_it = 0.5*t2_j + 0.5*t1_it; emitted late so the DVE
                # stream never stalls on the t2 roundtrip
                for it in range(NIT):
                    t12 = const.tile([P, M], F32, tag=f"t12_{it}")
                    nc.vector.tensor_scalar(out=t12, in0=t2r,
                                            scalar1=t1h[:, it:it + 1],
                                            scalar2=None, op0=OP.add)
                    t12s.append(t12)
            # first/last chunk: per-d ACT instructions to shorten the
            # pipeline ramp and the post-loop tail; middle chunks use one
            # wide instruction per (chunk, i-tile) for lower ACT overhead
            fine = ch in (0, NCH - 1)
            for it in range(NIT):
                sqr = (ch, it) in SQ_UNITS   # ACT-Square path (diff^2)
                sq = sqp.tile([P, G * M], F32)
                pieces = [(g * M, (g + 1) * M) for g in range(G)] if fine \
                    else [(0, G * M)]
                for g in range(G):
                    d = ch * G + g
                    if sqr:
                        nc.scalar.activation(
                            out=sq[:, g * M:(g + 1) * M],
                            in_=x2r[:, g * M:(g + 1) * M],
                            func=AF.Square,
                            bias=xneg[:, it * D + d:it * D + d + 1])
                        continue
                    nc.vector.tensor_scalar(
                        out=sq[:, g * M:(g + 1) * M],
                        in0=x2r[:, g * M:(g + 1) * M],
                        scalar1=xsb[:, it * D + d:it * D + d + 1],
                        scalar2=None, op0=OP.subtract)
                    if fine:
                        squ = sq[:, g * M:(g + 1) * M].bitcast(U32)
                        nc.vector.tensor_scalar(out=squ, in0=squ,
                                                scalar1=ABS_MASK, scalar2=None,
                                                op0=OP.bitwise_and)
                if not fine and not sqr:
                    # |diff| = clear the fp32 sign bit (uint32 view)
                    squ = sq[:, :].bitcast(U32)
                    nc.vector.tensor_scalar(out=squ, in0=squ,
                                            scalar1=ABS_MASK, scalar2=None,
                                            op0=OP.bitwise_and)
                for lo, hi in pieces:
                    nc.scalar.activation(out=sq[:, lo:hi], in_=sq[:, lo:hi],
                                         func=AF.Ln)
                    # path A: exp(p*ln|d| + ln.5); Square path: exp(H*ln(d^2) + ln.5)
                    nc.scalar.activation(out=sq[:, lo:hi], in_=sq[:, lo:hi],
                                         func=AF.Exp, bias=lnhalf[:, :],
                                         scale=hcol if sqr else pcol)
                eng = nc.gpsimd if (ch, it) in GPS_UNITS else nc.vector
                tgt = gaccs[it] if (ch, it) in GPS_UNITS else accs[it]
                for g in range(G):
                    if (ch, it) not in GPS_UNITS and it not in acc_started:
                        nc.vector.tensor_copy(tgt[:, :],
                                              sq[:, g * M:(g + 1) * M])
                        acc_started.add(it)
                        continue
                    eng.tensor_tensor(out=tgt, in0=tgt,
                                      in1=sq[:, g * M:(g + 1) * M], op=OP.add)

        # ---- merge accumulators, K = (0.5*t2_j + 0.5*t1_i) - acc, write.
        # it=0's chain runs on GpSimd in parallel with it=1's on DVE so the
        # post-loop tail is not one serial DVE chain. ----
        for it in range(NIT):
            if it in gaccs:
                nc.vector.tensor_tensor(out=accs[it], in0=accs[it],
                                        in1=gaccs[it], op=OP.add)
            nc.vector.tensor_tensor(out=accs[it], in0=t12s[it], in1=accs[it],
                                    op=OP.subtract)
            nc.sync.dma_start(out=out_ap[it * P:(it + 1) * P, :],
                               in_=accs[it])


def _get_nc(reps=1, body_reps=1):
    key = ("nc", reps, body_reps)
    if key not in _CACHE:
        _CACHE[key] = _build_nc(reps, body_reps)
    return _CACHE[key]


def _make_in_maps(x, X2, log_H):
    x = np.ascontiguousarray(np.asarray(x, dtype=np.float32))
    X2 = np.ascontiguousarray(np.asarray(X2, dtype=np.float32))
    logh = np.asarray(log_H, dtype=np.float32).reshape(1, 1)
    x2t = np.ascontiguousarray(X2.T)
    return [
        {"xsh": x[c * NS:(c + 1) * NS], "x2t": x2t, "x2n": X2, "logh": logh}
        for c in range(NCORES)
    ]


def run_spmd(x, X2, log_H, trace=False, reps=1, body_reps=1, **kw):
    nc = _get_nc(reps, body_reps)
    in_maps = _make_in_maps(x, X2, log_H)
    return run_bass_kernel_spmd(nc, in_maps, list(range(NCORES)),
                                trace=trace, **kw)


def kernel(x, X2, log_H):
    res = run_spmd(x, X2, log_H)
    return np.concatenate([res.results[c]["out"] for c in range(NCORES)], axis=0)


# revision 38
# speedup vs baseline: 1.0142x; 1.0142x over previous
"""Trainium2 Bass kernel: fractional Brownian motion kernel layer.

K[i,j] = 0.5 * sum_d (|x_id|^p + |X2_jd|^p - |x_id - X2_jd|^p),
p = 2*softplus(log_H),  x:[2048,16], X2:[2048,16] -> K:[2048,2048] f32.

Sharding: rows of x across 8 NeuronCores (256 rows each), X2 replicated.
Per-core layout: partition = i (2 tiles of 128), free = j (2048).

Pairwise pipeline per (chunk of 2 d's, i-tile):
  DVE : diff = X2r_d - x_col      tensor_scalar subtract @ 2x fp32 (per d)
  DVE : |diff|                    tensor_scalar bitwise_and 0x7FFFFFFF (uint32 view) @ 2x
  ACT : Ln                        one instr over the whole chunk
  ACT : Exp(scale=p, bias=ln.5)   -> 0.5*|diff|^p   (the 0.5 folds into the bias)
  DVE : acc += pw                 tensor_tensor add (first term is a copy)

Accumulators are zero-started; K = (0.5*t1_i + 0.5*t2_j) - acc is folded in
at the very end so nothing ever waits on the t2 DRAM roundtrip. First/last
chunks run per-d ACT instructions to shorten ramp and tail. GpSimd is kept
OFF the streaming path (it shares SBUF ports with DVE; HW-measured net loss)
and the scalar engine issues no DMAs (they stall its sequencer).

All activation functions used (Exp, Ln, Abs) live in the
natural_log_exp_and_others table set; get_activation_tables is narrowed so
bacc's table-load pass picks that single set (otherwise it ping-pongs
between exp_and_others and natural_log, reloading tables 19x per launch).
"""

from contextlib import ExitStack

import numpy as np

import concourse.bass as bass
import concourse.tile as tile
from concourse import mybir, bacc
from concourse.bass_utils import run_bass_kernel_spmd

AF = mybir.ActivationFunctionType
OP = mybir.AluOpType
AX = mybir.AxisListType
F32 = mybir.dt.float32
U32 = mybir.dt.uint32

N, M, D = 2048, 2048, 16
NCORES = 8
NS = N // NCORES          # 256 rows of x per core
P = 128                   # SBUF partitions
NIT = NS // P             # 2 i-tiles per core
G = 2                     # d's per chunk
NCH = D // G              # chunks
# (chunk, i-tile) units whose accumulation runs on GpSimd instead of DVE,
# spread across the schedule so DVE never starves on sq-slot reuse
GPS_UNITS = set()   # GpSimd streaming contends with DVE SBUF ports (HW-measured)
# units whose |diff| is computed as ACT Square(x2r - x) instead of DVE sub+mask,
# balancing DVE vs ACT load
SQ_UNITS = set()    # ACT is the HW-binding engine; keep all abs work on DVE
LN_HALF = float(np.log(0.5))
ABS_MASK = 0x7FFFFFFF

_CACHE = {}


def _patch_act_tables():
    """Force every activation function we use into one table set so the
    act-table-load pass emits a single load."""
    if _CACHE.get("patched"):
        return
    import concourse.hw_specs as hw_specs
    import concourse.bacc as bacc_mod

    orig = hw_specs.get_activation_tables
    ours = {AF.Exp, AF.Ln, AF.Abs, AF.Square}

    def patched(module_arch):
        tabs = {k: set(v) for k, v in orig(module_arch).items()}
        for name, fns in tabs.items():
            if name != "natural_log_exp_and_others":
                fns -= ours
        return tabs

    bacc_mod.get_activation_tables = patched
    _CACHE["patched"] = True


def _build_nc(reps=1, body_reps=1):
    _patch_act_tables()
    nc = bacc.Bacc(trn_type="TRN2", target_bir_lowering=False, debug=False,
                   num_devices=NCORES)

    xsh = nc.declare_dram_parameter("xsh", [NS, D], F32, isOutput=False)
    x2t = nc.declare_dram_parameter("x2t", [D, M], F32, isOutput=False)
    x2n = nc.declare_dram_parameter("x2n", [M, D], F32, isOutput=False)
    logh = nc.declare_dram_parameter("logh", [1, 1], F32, isOutput=False)
    out = nc.declare_dram_parameter("out", [NS, M], F32, isOutput=True)
    t2scr = nc.dram_tensor("t2scr", [1, M], F32)

    xsh_ap, x2t_ap, x2n_ap, logh_ap, out_ap, t2scr_ap = (
        h.ap() for h in (xsh, x2t, x2n, logh, out, t2scr))

    with tile.TileContext(nc) as tc, ExitStack() as ctx:
        const = ctx.enter_context(tc.tile_pool(name="const", bufs=1))
        x2rp = ctx.enter_context(tc.tile_pool(name="x2r", bufs=3))
        sqp = ctx.enter_context(tc.tile_pool(name="sq", bufs=5))
        accp = ctx.enter_context(tc.tile_pool(name="acc", bufs=1))

        if reps > 1:  # benchmark mode: repeat the whole body on-device
            loop = ctx.enter_context(
                tc.For_i(0, reps, 1, staggered_reset=True))

        for _body in range(body_reps):
            _emit_body(nc, tc, const, x2rp, sqp, accp,
                       xsh_ap, x2t_ap, x2n_ap, logh_ap, out_ap, t2scr_ap)

    nc.compile()
    return nc


def _emit_body(nc, tc, const, x2rp, sqp, accp,
               xsh_ap, x2t_ap, x2n_ap, logh_ap, out_ap, t2scr_ap):
    if True:

        # ---- tiny latency-critical loads first (sync/SP HWDGE ring) ----
        # x shard -> [128, it*16+d] (one DMA; one completion sem)
        xsb = const.tile([P, NIT * D], F32)
        nc.sync.dma_start(
            out=xsb,
            in_=bass.AP(tensor=xsh_ap.tensor, offset=0,
                        ap=[[D, P], [P * D, NIT], [1, D]]))

        if SQ_UNITS:
            xneg = const.tile([P, NIT * D], F32)
            nc.vector.tensor_scalar(out=xneg[:, :], in0=xsb[:, :],
                                    scalar1=-1.0, scalar2=None, op0=OP.mult)

        lh = const.tile([P, 1], F32)
        nc.sync.dma_start(
            out=lh,
            in_=bass.AP(tensor=logh_ap.tensor, offset=0, ap=[[0, P], [1, 1]]))

        # ---- X2 broadcast rows, one 1MB DMA per d for fine pipelining;
        # chunk 0 first so compute can start immediately ----
        x2rs = {}
        def load_chunk(ch):
            x2r = x2rp.tile([P, G * M], F32)
            for g in range(G):
                d = ch * G + g
                nc.sync.dma_start(
                    out=x2r[:, g * M:(g + 1) * M],
                    in_=bass.AP(tensor=x2t_ap.tensor, offset=d * M,
                                ap=[[0, P], [1, M]]))
            x2rs[ch] = x2r
        load_chunk(0)
        x2c = const.tile([P, (M // P) * D], F32)
        JT = M // P   # 16 j's per partition, p-major: j = p*JT + jt
        nc.sync.dma_start(
            out=x2c,
            in_=bass.AP(tensor=x2n_ap.tensor, offset=0,
                        ap=[[JT * D, P], [D, JT], [1, D]]))
        for ch in range(1, NCH):
            load_chunk(ch)

        # ---- H and p = 2H, replicated to all partitions ----
        eh = const.tile([P, 1], F32)
        nc.scalar.activation(out=eh, in_=lh, func=AF.Exp)
        hcol = const.tile([P, 1], F32)
        # ln(1 + e^logh) = softplus(logh) = H
        nc.scalar.activation(out=hcol, in_=eh, func=AF.Ln, bias=1.0)
        pcol = const.tile([P, 1], F32)
        nc.scalar.mul(pcol, hcol, 2.0)
        lnhalf = const.tile([P, 1], F32)
        nc.gpsimd.memset(lnhalf[:, :], LN_HALF)

        # ---- t1 (halved): 0.5*sum_d |x_id|^p, per-partition scalars ----
        e1 = const.tile([P, NIT * D], F32)
        nc.scalar.activation(out=e1, in_=xsb, func=AF.Abs)
        nc.scalar.activation(out=e1, in_=e1, func=AF.Ln)
        nc.scalar.activation(out=e1, in_=e1, func=AF.Exp,
                             bias=lnhalf[:, :], scale=pcol)
        t1h = const.tile([P, NIT], F32)
        nc.vector.tensor_reduce(
            out=t1h[:, :], in_=e1[:, :].rearrange("p (it d) -> p it d", it=NIT),
            axis=AX.X, op=OP.add)

        # ---- t2 (halved): compact [128 j, 16 jt x 16 d] -> row of 2048 ----
        e2 = const.tile([P, (M // P) * D], F32)
        nc.scalar.activation(out=e2, in_=x2c, func=AF.Abs)
        nc.scalar.activation(out=e2, in_=e2, func=AF.Ln)
        nc.scalar.activation(out=e2, in_=e2, func=AF.Exp,
                             bias=lnhalf[:, :], scale=pcol)
        t2c = const.tile([P, M // P], F32)
        nc.vector.tensor_reduce(
            out=t2c[:, :], in_=e2[:, :].rearrange("p (jt d) -> p jt d", d=D),
            axis=AX.X, op=OP.add)
        # roundtrip through DRAM to rearrange [p, jt] -> row j = jt*128+p,
        # then broadcast that row across all 128 partitions. Issued on the
        # scalar engine's HWDGE ring so it never blocks the x2r stream on
        # the sync ring (HWDGE DMAs are FIFO per issuing engine).
        nc.sync.dma_start(
            out=bass.AP(tensor=t2scr_ap.tensor, offset=0,
                        ap=[[JT, P], [1, JT]]),
            in_=t2c)
        t2r = const.tile([P, M], F32)
        nc.sync.dma_start(
            out=t2r,
            in_=bass.AP(tensor=t2scr_ap.tensor, offset=0, ap=[[0, P], [1, M]]))

        # ---- accumulators: zero-init (no dependency on the t2 roundtrip;
        # t1+t2 is folded in at the very end) ----
        accs, gaccs = [], {}
        acc_started = set()
        for it in range(NIT):
            acc = accp.tile([P, M], F32, tag=f"acc{it}")
            accs.append(acc)
            if any(i2 == it for (_c, i2) in GPS_UNITS):
                gacc = accp.tile([P, M], F32, tag=f"gacc{it}")
                nc.gpsimd.memset(gacc[:, :], 0.0)
                gaccs[it] = gacc

        # ---- main loop: acc += 0.5*|x_id - X2_jd|^p ----
        t12s = []
        for ch in range(NCH):
            x2r = x2rs[ch]
            if ch == NCH - 1:
                # t12_it = 0.5*t2_j + 0.5*t1_it; emitted late so the DVE
                # stream never stalls on the t2 roundtrip
                for it in range(NIT):
                    t12 = const.tile([P, M], F32, tag=f"t12_{it}")
                    nc.vector.tensor_scalar(out=t12, in0=t2r,
                                            scalar1=t1h[:, it:it + 1],
                                            scalar2=None, op0=OP.add)
                    t12s.append(t12)
            # first/last chunk: per-d ACT instructions to shorten the
            # pipeline ramp and the post-loop tail; middle chunks use one
            # wide instruction per (chunk, i-tile) for lower ACT overhead
            fine = ch in (0, NCH - 1)
            for it in range(NIT):
                sqr = (ch, it) in SQ_UNITS   # ACT-Square path (diff^2)
                sq = sqp.tile([P, G * M], F32)
                pieces = [(g * M, (g + 1) * M) for g in range(G)] if fine \
                    else [(0, G * M)]
                for g in range(G):
                    d = ch * G + g
                    if sqr:
                        nc.scalar.activation(
                            out=sq[:, g * M:(g + 1) * M],
                            in_=x2r[:, g * M:(g + 1) * M],
                            func=AF.Square,
                            bias=xneg[:, it * D + d:it * D + d + 1])
                        continue
                    nc.vector.tensor_scalar(
                        out=sq[:, g * M:(g + 1) * M],
                        in0=x2r[:, g * M:(g + 1) * M],
                        scalar1=xsb[:, it * D + d:it * D + d + 1],
                        scalar2=None, op0=OP.subtract)
                    if fine:
                        squ = sq[:, g * M:(g + 1) * M].bitcast(U32)
                        nc.vector.tensor_scalar(out=squ, in0=squ,
                                                scalar1=ABS_MASK, scalar2=None,
                                                op0=OP.bitwise_and)
                if not fine and not sqr:
                    # |diff| = clear the fp32 sign bit (uint32 view)
                    squ = sq[:, :].bitcast(U32)
                    nc.vector.tensor_scalar(out=squ, in0=squ,
                                            scalar1=ABS_MASK, scalar2=None,
                                            op0=OP.bitwise_and)
                for lo, hi in pieces:
                    nc.scalar.activation(out=sq[:, lo:hi], in_=sq[:, lo:hi],
                                         func=AF.Ln)
                    # path A: exp(p*ln|d| + ln.5); Square path: exp(H*ln(d^2) + ln.5)
                    nc.scalar.activation(out=sq[:, lo:hi], in_=sq[:, lo:hi],
                                         func=AF.Exp, bias=lnhalf[:, :],
                                         scale=hcol if sqr else pcol)
                eng = nc.gpsimd if (ch, it) in GPS_UNITS else nc.vector
                tgt = gaccs[it] if (ch, it) in GPS_UNITS else accs[it]
                for g in range(G):
                    if (ch, it) not in GPS_UNITS and it not in acc_started:
                        nc.vector.tensor_copy(tgt[:, :],
                                              sq[:, g * M:(g + 1) * M])
                        acc_started.add(it)
                        continue
                    eng.tensor_tensor(out=tgt, in0=tgt,
                                      in1=sq[:, g * M:(g + 1) * M], op=OP.add)

        # ---- merge accumulators, K = (0.5*t2_j + 0.5*t1_i) - acc, write.
        # it=0's chain runs on GpSimd in parallel with it=1's on DVE so the
        # post-loop tail is not one serial DVE chain. ----
        for it in range(NIT):
            if it in gaccs:
                nc.vector.tensor_tensor(out=accs[it], in0=accs[it],
                                        in1=gaccs[it], op=OP.add)
            nc.vector.tensor_tensor(out=accs[it], in0=t12s[it], in1=accs[it],
                                    op=OP.subtract)
            nc.sync.dma_start(out=out_ap[it * P:(it + 1) * P, :],
                               in_=accs[it])


def _get_nc(reps=1, body_reps=1):
    key = ("nc", reps, body_reps)
    if key not in _CACHE:
        _CACHE[key] = _build_nc(reps, body_reps)
    return _CACHE[key]


def _make_in_maps(x, X2, log_H):
    x = np.ascontiguousarray(np.asarray(x, dtype=np.float32))
    X2 = np.ascontiguousarray(np.asarray(X2, dtype=np.float32))
    logh = np.asarray(log_H, dtype=np.float32).reshape(1, 1)
    x2t = np.ascontiguousarray(X2.T)
    return [
        {"xsh": x[c * NS:(c + 1) * NS], "x2t": x2t, "x2n": X2, "logh": logh}
        for c in range(NCORES)
    ]


def run_spmd(x, X2, log_H, trace=False, reps=1, body_reps=1, **kw):
    nc = _get_nc(reps, body_reps)
    in_maps = _make_in_maps(x, X2, log_H)
    return run_bass_kernel_spmd(nc, in_maps, list(range(NCORES)),
                                trace=trace, **kw)


def kernel(x, X2, log_H):
    res = run_spmd(x, X2, log_H)
    return np.concatenate([res.results[c]["out"] for c in range(NCORES)], axis=0)


# revision 41
# speedup vs baseline: 1.0952x; 1.0798x over previous
"""Trainium2 Bass kernel: fractional Brownian motion kernel layer.

K[i,j] = 0.5 * sum_d (|x_id|^p + |X2_jd|^p - |x_id - X2_jd|^p),
p = 2*softplus(log_H),  x:[2048,16], X2:[2048,16] -> K:[2048,2048] f32.

Sharding: rows of x across 8 NeuronCores (256 rows each), X2 replicated.
Per-core layout: partition = i (2 tiles of 128), free = j (2048).

Pairwise pipeline per (chunk of 2 d's, i-tile):
  DVE : diff = X2r_d - x_col      tensor_scalar subtract @ 2x fp32 (per d)
  DVE : |diff|                    tensor_scalar bitwise_and 0x7FFFFFFF (uint32 view) @ 2x
  ACT : Ln                        one instr over the whole chunk
  ACT : Exp(scale=p, bias=ln.5)   -> 0.5*|diff|^p   (the 0.5 folds into the bias)
  DVE : acc += pw                 tensor_tensor add (first term is a copy)

Accumulators are zero-started; K = (0.5*t1_i + 0.5*t2_j) - acc is folded in
at the very end so nothing ever waits on the t2 DRAM roundtrip. First/last
chunks run per-d ACT instructions to shorten ramp and tail. GpSimd is kept
OFF the streaming path (it shares SBUF ports with DVE; HW-measured net loss)
and the scalar engine issues no DMAs (they stall its sequencer).

All activation functions used (Exp, Ln, Abs) live in the
natural_log_exp_and_others table set; get_activation_tables is narrowed so
bacc's table-load pass picks that single set (otherwise it ping-pongs
between exp_and_others and natural_log, reloading tables 19x per launch).
"""

from contextlib import ExitStack

import numpy as np

import concourse.bass as bass
import concourse.tile as tile
from concourse import mybir, bacc
from concourse.bass_utils import run_bass_kernel_spmd

AF = mybir.ActivationFunctionType
OP = mybir.AluOpType
AX = mybir.AxisListType
F32 = mybir.dt.float32
U32 = mybir.dt.uint32

N, M, D = 2048, 2048, 16
NCORES = 8
NS = N // NCORES          # 256 rows of x per core
P = 128                   # SBUF partitions
NIT = NS // P             # 2 i-tiles per core
G = 2                     # d's per chunk
NCH = D // G              # chunks
# (chunk, i-tile) units whose accumulation runs on GpSimd instead of DVE,
# spread across the schedule so DVE never starves on sq-slot reuse
GPS_UNITS = set()   # GpSimd streaming contends with DVE SBUF ports (HW-measured)
# units whose |diff| is computed as ACT Square(x2r - x) instead of DVE sub+mask,
# balancing DVE vs ACT load
SQ_UNITS = set()    # ACT is the HW-binding engine; keep all abs work on DVE
LN_HALF = float(np.log(0.5))
ABS_MASK = 0x7FFFFFFF

_CACHE = {}


def _patch_act_tables():
    """Force every activation function we use into one table set so the
    act-table-load pass emits a single load."""
    if _CACHE.get("patched"):
        return
    import concourse.hw_specs as hw_specs
    import concourse.bacc as bacc_mod

    orig = hw_specs.get_activation_tables
    ours = {AF.Exp, AF.Ln, AF.Abs, AF.Square}

    def patched(module_arch):
        tabs = {k: set(v) for k, v in orig(module_arch).items()}
        for name, fns in tabs.items():
            if name != "natural_log_exp_and_others":
                fns -= ours
        return tabs

    bacc_mod.get_activation_tables = patched
    _CACHE["patched"] = True


def _build_nc(reps=1, body_reps=1):
    _patch_act_tables()
    nc = bacc.Bacc(trn_type="TRN2", target_bir_lowering=False, debug=False,
                   num_devices=NCORES)

    xsh = nc.declare_dram_parameter("xsh", [NS, D], F32, isOutput=False)
    x2t = nc.declare_dram_parameter("x2t", [D, M], F32, isOutput=False)
    x2n = nc.declare_dram_parameter("x2n", [M, D], F32, isOutput=False)
    logh = nc.declare_dram_parameter("logh", [1, 1], F32, isOutput=False)
    out = nc.declare_dram_parameter("out", [NS, M], F32, isOutput=True)
    t2scr = nc.dram_tensor("t2scr", [1, M], F32)

    xsh_ap, x2t_ap, x2n_ap, logh_ap, out_ap, t2scr_ap = (
        h.ap() for h in (xsh, x2t, x2n, logh, out, t2scr))

    with tile.TileContext(nc) as tc, ExitStack() as ctx:
        const = ctx.enter_context(tc.tile_pool(name="const", bufs=1))
        x2rp = ctx.enter_context(tc.tile_pool(name="x2r", bufs=4))
        sqp = ctx.enter_context(tc.tile_pool(name="sq", bufs=5))
        accp = ctx.enter_context(tc.tile_pool(name="acc", bufs=1))

        if reps > 1:  # benchmark mode: repeat the whole body on-device
            loop = ctx.enter_context(
                tc.For_i(0, reps, 1, staggered_reset=True))

        for _body in range(body_reps):
            _emit_body(nc, tc, const, x2rp, sqp, accp,
                       xsh_ap, x2t_ap, x2n_ap, logh_ap, out_ap, t2scr_ap)

    nc.compile()
    return nc


def _emit_body(nc, tc, const, x2rp, sqp, accp,
               xsh_ap, x2t_ap, x2n_ap, logh_ap, out_ap, t2scr_ap):
    if True:

        # ---- tiny latency-critical loads first (sync/SP HWDGE ring) ----
        # x shard -> [128, it*16+d] (one DMA; one completion sem)
        xsb = const.tile([P, NIT * D], F32)
        nc.sync.dma_start(
            out=xsb,
            in_=bass.AP(tensor=xsh_ap.tensor, offset=0,
                        ap=[[D, P], [P * D, NIT], [1, D]]))

        if SQ_UNITS:
            xneg = const.tile([P, NIT * D], F32)
            nc.vector.tensor_scalar(out=xneg[:, :], in0=xsb[:, :],
                                    scalar1=-1.0, scalar2=None, op0=OP.mult)

        lh = const.tile([P, 1], F32)
        nc.sync.dma_start(
            out=lh,
            in_=bass.AP(tensor=logh_ap.tensor, offset=0, ap=[[0, P], [1, 1]]))

        # ---- X2 broadcast rows, one 1MB DMA per d for fine pipelining;
        # chunk 0 first so compute can start immediately ----
        x2rs = {}
        def load_chunk(ch):
            x2r = x2rp.tile([P, G * M], F32)
            for g in range(G):
                d = ch * G + g
                nc.sync.dma_start(
                    out=x2r[:, g * M:(g + 1) * M],
                    in_=bass.AP(tensor=x2t_ap.tensor, offset=d * M,
                                ap=[[0, P], [1, M]]))
            x2rs[ch] = x2r
        load_chunk(0)
        x2c = const.tile([P, (M // P) * D], F32)
        JT = M // P   # 16 j's per partition, p-major: j = p*JT + jt
        nc.sync.dma_start(
            out=x2c,
            in_=bass.AP(tensor=x2n_ap.tensor, offset=0,
                        ap=[[JT * D, P], [D, JT], [1, D]]))
        for ch in range(1, NCH):
            load_chunk(ch)

        # ---- H and p = 2H, replicated to all partitions ----
        eh = const.tile([P, 1], F32)
        nc.scalar.activation(out=eh, in_=lh, func=AF.Exp)
        hcol = const.tile([P, 1], F32)
        # ln(1 + e^logh) = softplus(logh) = H
        nc.scalar.activation(out=hcol, in_=eh, func=AF.Ln, bias=1.0)
        pcol = const.tile([P, 1], F32)
        nc.scalar.mul(pcol, hcol, 2.0)
        lnhalf = const.tile([P, 1], F32)
        nc.gpsimd.memset(lnhalf[:, :], LN_HALF)

        # ---- t1 (halved): 0.5*sum_d |x_id|^p, per-partition scalars ----
        e1 = const.tile([P, NIT * D], F32)
        nc.scalar.activation(out=e1, in_=xsb, func=AF.Abs)
        nc.scalar.activation(out=e1, in_=e1, func=AF.Ln)
        nc.scalar.activation(out=e1, in_=e1, func=AF.Exp,
                             bias=lnhalf[:, :], scale=pcol)
        t1h = const.tile([P, NIT], F32)
        nc.vector.tensor_reduce(
            out=t1h[:, :], in_=e1[:, :].rearrange("p (it d) -> p it d", it=NIT),
            axis=AX.X, op=OP.add)

        # ---- t2 (halved): compact [128 j, 16 jt x 16 d] -> row of 2048 ----
        e2 = const.tile([P, (M // P) * D], F32)
        nc.scalar.activation(out=e2, in_=x2c, func=AF.Abs)
        nc.scalar.activation(out=e2, in_=e2, func=AF.Ln)
        nc.scalar.activation(out=e2, in_=e2, func=AF.Exp,
                             bias=lnhalf[:, :], scale=pcol)
        t2c = const.tile([P, M // P], F32)
        nc.vector.tensor_reduce(
            out=t2c[:, :], in_=e2[:, :].rearrange("p (jt d) -> p jt d", d=D),
            axis=AX.X, op=OP.add)
        # roundtrip through DRAM to rearrange [p, jt] -> row j = jt*128+p,
        # then broadcast that row across all 128 partitions. Issued on the
        # scalar engine's HWDGE ring so it never blocks the x2r stream on
        # the sync ring (HWDGE DMAs are FIFO per issuing engine).
        nc.sync.dma_start(
            out=bass.AP(tensor=t2scr_ap.tensor, offset=0,
                        ap=[[JT, P], [1, JT]]),
            in_=t2c)
        t2r = const.tile([P, M], F32)
        nc.sync.dma_start(
            out=t2r,
            in_=bass.AP(tensor=t2scr_ap.tensor, offset=0, ap=[[0, P], [1, M]]))

        # ---- accumulators: zero-init (no dependency on the t2 roundtrip;
        # t1+t2 is folded in at the very end) ----
        accs, gaccs = [], {}
        acc_started = set()
        for it in range(NIT):
            acc = accp.tile([P, M], F32, tag=f"acc{it}")
            accs.append(acc)
            if any(i2 == it for (_c, i2) in GPS_UNITS):
                gacc = accp.tile([P, M], F32, tag=f"gacc{it}")
                nc.gpsimd.memset(gacc[:, :], 0.0)
                gaccs[it] = gacc

        # ---- main loop: acc += 0.5*|x_id - X2_jd|^p ----
        t12s = []
        for ch in range(NCH):
            x2r = x2rs[ch]
            if ch == NCH - 1:
                # t12_it = 0.5*t2_j + 0.5*t1_it; emitted late so the DVE
                # stream never stalls on the t2 roundtrip
                for it in range(NIT):
                    t12 = const.tile([P, M], F32, tag=f"t12_{it}")
                    nc.vector.tensor_scalar(out=t12, in0=t2r,
                                            scalar1=t1h[:, it:it + 1],
                                            scalar2=None, op0=OP.add)
                    t12s.append(t12)
            # first/last chunk: per-d ACT instructions to shorten the
            # pipeline ramp and the post-loop tail; middle chunks use one
            # wide instruction per (chunk, i-tile) for lower ACT overhead
            fine = ch in (0, NCH - 1)
            for it in range(NIT):
                sqr = (ch, it) in SQ_UNITS   # ACT-Square path (diff^2)
                sq = sqp.tile([P, G * M], F32)
                pieces = [(g * M, (g + 1) * M) for g in range(G)] if fine \
                    else [(0, G * M)]
                for g in range(G):
                    d = ch * G + g
                    if sqr:
                        nc.scalar.activation(
                            out=sq[:, g * M:(g + 1) * M],
                            in_=x2r[:, g * M:(g + 1) * M],
                            func=AF.Square,
                            bias=xneg[:, it * D + d:it * D + d + 1])
                        continue
                    nc.vector.tensor_scalar(
                        out=sq[:, g * M:(g + 1) * M],
                        in0=x2r[:, g * M:(g + 1) * M],
                        scalar1=xsb[:, it * D + d:it * D + d + 1],
                        scalar2=None, op0=OP.subtract)
                    if fine:
                        squ = sq[:, g * M:(g + 1) * M].bitcast(U32)
                        nc.vector.tensor_scalar(out=squ, in0=squ,
                                                scalar1=ABS_MASK, scalar2=None,
                                                op0=OP.bitwise_and)
                if not fine and not sqr:
                    # |diff| = clear the fp32 sign bit (uint32 view)
                    squ = sq[:, :].bitcast(U32)
                    nc.vector.tensor_scalar(out=squ, in0=squ,
                                            scalar1=ABS_MASK, scalar2=None,
                                            op0=OP.bitwise_and)
                for lo, hi in pieces:
                    nc.scalar.activation(out=sq[:, lo:hi], in_=sq[:, lo:hi],
                                         func=AF.Ln)
                    # path A: exp(p*ln|d| + ln.5); Square path: exp(H*ln(d^2) + ln.5)
                    nc.scalar.activation(out=sq[:, lo:hi], in_=sq[:, lo:hi],
                                         func=AF.Exp, bias=lnhalf[:, :],
                                         scale=hcol if sqr else pcol)
                eng = nc.gpsimd if (ch, it) in GPS_UNITS else nc.vector
                tgt = gaccs[it] if (ch, it) in GPS_UNITS else accs[it]
                for g in range(G):
                    if (ch, it) not in GPS_UNITS and it not in acc_started:
                        nc.vector.tensor_copy(tgt[:, :],
                                              sq[:, g * M:(g + 1) * M])
                        acc_started.add(it)
                        continue
                    eng.tensor_tensor(out=tgt, in0=tgt,
                                      in1=sq[:, g * M:(g + 1) * M], op=OP.add)

        # ---- merge accumulators, K = (0.5*t2_j + 0.5*t1_i) - acc, write.
        # it=0's chain runs on GpSimd in parallel with it=1's on DVE so the
        # post-loop tail is not one serial DVE chain. ----
        for it in range(NIT):
            if it in gaccs:
                nc.vector.tensor_tensor(out=accs[it], in0=accs[it],
                                        in1=gaccs[it], op=OP.add)
            nc.vector.tensor_tensor(out=accs[it], in0=t12s[it], in1=accs[it],
                                    op=OP.subtract)
            nc.sync.dma_start(out=out_ap[it * P:(it + 1) * P, :],
                               in_=accs[it])


def _get_nc(reps=1, body_reps=1):
    key = ("nc", reps, body_reps)
    if key not in _CACHE:
        _CACHE[key] = _build_nc(reps, body_reps)
    return _CACHE[key]


def _make_in_maps(x, X2, log_H):
    x = np.ascontiguousarray(np.asarray(x, dtype=np.float32))
    X2 = np.ascontiguousarray(np.asarray(X2, dtype=np.float32))
    logh = np.asarray(log_H, dtype=np.float32).reshape(1, 1)
    x2t = np.ascontiguousarray(X2.T)
    return [
        {"xsh": x[c * NS:(c + 1) * NS], "x2t": x2t, "x2n": X2, "logh": logh}
        for c in range(NCORES)
    ]


def run_spmd(x, X2, log_H, trace=False, reps=1, body_reps=1, **kw):
    nc = _get_nc(reps, body_reps)
    in_maps = _make_in_maps(x, X2, log_H)
    return run_bass_kernel_spmd(nc, in_maps, list(range(NCORES)),
                                trace=trace, **kw)


def kernel(x, X2, log_H):
    res = run_spmd(x, X2, log_H)
    return np.concatenate([res.results[c]["out"] for c in range(NCORES)], axis=0)
